# revision 11
# baseline (speedup 1.0000x reference)
"""Trainium2 Bass kernel for nn_BasicBlock (3-layer GCN block with residual).

Math (per batch item b, per conv):
    out = A @ (x @ W) + bias,  A = normalized adjacency (with self loops)
where A[c, r] = sum over edges r->c of dinv[r]*dinv[c] (dense N x N, shared
across batch and precomputed on host from the edge lists).

Block:
    a1 = relu(A_sp @ (x  @ W1) + b1)
    a2 = relu(A_tm @ (a1 @ W2) + b2)
    o3 =      A_sp @ (a2 @ W3) + b3
    out = relu(o3 + x)

On-chip layouts per item (P=128 partitions):
    natural  [n, c] : node chunks on partitions           (lhsT of A-matmuls)
    transposed [c, n]: channel chunks on partitions       (lhsT of W-matmuls)

Phases per item (matmul forms; AT = A^T so AT[m, n] = A[n, m]):
    1. g1T[c,n]  = sum_m x[m,c]  * AT_sp[m,n]      (lhsT=x chunk,  rhs=AT_sp)
    2. a1T[co,n] = relu(sum_ci W1[ci,co]*g1T[ci,n] + b1)   (lhsT=W1, rhs=g1T)
    3. h2[n,c]   = sum_ci a1T[ci,n] * W2[ci,c]     (lhsT=a1T chunk, rhs=W2)
    4. a2T[c,n]  = relu(sum_m h2[m,c]*AT_tm[m,n] + b2)
    5. h3[n,c]   = sum_ci a2T[ci,n] * W3[ci,c];  h3[N,:] = b3
    6. outT[c,n] = relu(sum_m h3[m,c]*AT_sp[m,n] + xT[c,n])
       (transposed-output form, same shape as phase 1: lhsT = h3 chunks,
        rhs = AT_sp streamed F=1700 -- amortizes ldweights 4x better than
        the natural-output form and stores out as [c, n]; the host
        transposes back.  AT_sp row N is all-ones over valid cols -> adds
        b3 to every node; harmless in phase 1 because x row N is
        zero-padded.)

All matmuls run in fp8 e4m3 with DoubleRow perf mode (two 128-deep k-tiles
contracted per instruction) and fp32 PSUM accumulation. The conv-path signal
is tiny relative to the identity residual, so fp8 error washes out; the
residual itself stays bf16 (x arrives from host as fp8 natural [n,c] for
phase 1 and bf16 transposed [c,n] for the phase-6 residual).
Batch (64) is sharded 8 items/core over the 8 cores; A/W/b are replicated.
"""

import sys

if "/opt/trn_rl_repo" not in sys.path:
    sys.path.insert(0, "/opt/trn_rl_repo")

import numpy as np
import ml_dtypes

import concourse.bass as bass
import concourse.bacc as bacc
import concourse.mybir as mybir
import concourse.tile as tile
from concourse.bass_utils import run_bass_kernel_spmd

P = 128
B, N, C = 64, 1700, 256
N_CORES = 8
B_LOCAL = B // N_CORES

F32 = mybir.dt.float32
BF16 = mybir.dt.bfloat16
F8 = mybir.dt.float8e4
RELU = mybir.ActivationFunctionType.Relu
DR = mybir.MatmulPerfMode.DoubleRow
NP_BF16 = ml_dtypes.bfloat16
NP_F8 = ml_dtypes.float8_e4m3


def _quarters(total, step=512):
    return [(q, min(step, total - q)) for q in range(0, total, step)]


def build_program(bl, n, c):
    """Build the Bass/Tile program for `bl` batch items, `n` nodes, `c` chans."""
    kt = -(-(n + 1) // P)  # node chunks; >= one pad row (bias row at index n)
    assert kt % 2 == 0, "DoubleRow pairing needs an even k-tile count"
    kp = kt // 2
    npad = kt * P
    npr = -(-n // 16) * 16  # at-tile row pitch: DoubleRow needs 16B-aligned strides
    ct = c // P

    nqv = _quarters(n)  # valid-column quarters (pads are never read)
    # phase-1 / at_sp-load order: small quarter first so the very first
    # matmul's operands (42KB of at_sp) land as early as possible
    nqv0 = nqv[-1:] + nqv[:-1]

    nc = bacc.Bacc("TRN2", target_bir_lowering=False, debug=False,
                   enable_asserts=False)

    x8_d = nc.dram_tensor("x8", [bl, npad, c], F8, kind="ExternalInput")
    xt_d = nc.dram_tensor("xt", [bl, c, n], BF16, kind="ExternalInput")
    atsp_d = nc.dram_tensor("at_sp", [P, kt, npr], F8, kind="ExternalInput")
    attm_d = nc.dram_tensor("at_tm", [P, kt, npr], F8, kind="ExternalInput")
    w_d = [nc.dram_tensor(f"w{i}", [P, ct, c], F8, kind="ExternalInput")
           for i in (1, 2, 3)]
    b1_d = nc.dram_tensor("b1", [P, ct], F32, kind="ExternalInput")
    b2_d = nc.dram_tensor("b2", [P, ct], F32, kind="ExternalInput")
    b3_d = nc.dram_tensor("b3", [1, c], F8, kind="ExternalInput")
    id_d = nc.dram_tensor("ident", [P, P], BF16, kind="ExternalInput")
    out_d = nc.dram_tensor("out", [bl, c, n], BF16, kind="ExternalOutput")

    with tile.TileContext(nc) as tc:
        with (
            tc.tile_pool(name="const", bufs=1) as cpool,
            tc.tile_pool(name="xq", bufs=5) as xqp,
            tc.tile_pool(name="xt", bufs=4) as xtp,
            tc.tile_pool(name="act", bufs=5) as actp,
            tc.tile_pool(name="h", bufs=3) as hp,
            tc.tile_pool(name="outp", bufs=6) as outp,
            tc.tile_pool(name="psA", bufs=5, space="PSUM") as psA,
            tc.tile_pool(name="psW", bufs=3, space="PSUM") as psW,
        ):
            # --- constants.  Ring plan: x8_0's first pairs front-run
            # everything on the two fast HWDGE rings (they gate the first
            # matmuls), then at_sp quarter-by-quarter in exactly phase-1
            # consumption order (alternating rings), then w+b on scalar,
            # then at_tm behind those; the remaining x rides gpsimd / late
            # sync; out stores go on sync. ---
            at_sp = cpool.tile([P, kt, npr], F8, tag="at_sp")
            at_tm = cpool.tile([P, kt, npr], F8, tag="at_tm")
            w_sb = [cpool.tile([P, ct, c], F8, tag=f"w{i}", name=f"w{i}")
                    for i in range(3)]
            b1_sb = cpool.tile([P, ct], F32, tag="b1")
            b2_sb = cpool.tile([P, ct], F32, tag="b2")
            id_sb = cpool.tile([P, P], BF16, tag="ident")

            def emit_load_at(at, at_d, split):
                # (pair, quarter) granularity in phase-1/4 consumption
                # order. at_sp splits across both fast rings (it gates the
                # first matmuls); at_tm rides sync only so the scalar
                # engine's queue is free for the phase-2/3 drains that
                # start while at_tm is still loading.
                i = 0
                for k in range(kp):
                    for (q0, qs) in (nqv0 if split else nqv):
                        eng = nc.scalar if (split and i % 2) else nc.sync
                        eng.dma_start(at[:, 2 * k:2 * k + 2, q0:q0 + qs],
                                      at_d[:, 2 * k:2 * k + 2, q0:q0 + qs])
                        i += 1

            def emit_load_w_b():
                for w, wd in zip(w_sb, w_d):
                    nc.scalar.dma_start(w[:], wd[:])
                nc.scalar.dma_start(b1_sb[:], b1_d[:])
                nc.scalar.dma_start(b2_sb[:], b2_d[:])
                nc.scalar.dma_start(id_sb[:], id_d[:])

            bias_tile = n // P      # global node index n == first pad row
            bias_part = n % P

            def emit_load_x8(b):
                # fp8 x (phase-1 stationary), host-padded to npad rows so the
                # whole tile DMAs without memsets. Item 0 gates the very first
                # matmuls, so its first pair is split across the two fast
                # HWDGE rings ahead of at_sp; later items ride the idle
                # gpsimd SWDGE ring.
                x8 = xqp.tile([P, kt, c], F8, tag="xq", name=f"x8_{b}")
                for k in range(kt):
                    if b == 0 and k < 2:
                        eng = nc.sync if k % 2 == 0 else nc.scalar
                    elif b <= 1:
                        eng = nc.gpsimd
                    else:
                        eng = nc.sync
                    eng.dma_start(x8[:, k, :], x8_d[b, k * P:(k + 1) * P, :])
                return x8

            def emit_load_xt(b):
                # bf16 transposed residual, needed only at phase 6 and only
                # 2 descriptors per item: rides the slow gpsimd SWDGE ring
                # so the fast rings carry the earlier-needed x8 + stores
                eng = nc.gpsimd
                xt = xtp.tile([P, ct, n], BF16, tag="xt", name=f"xt_{b}")
                for cc in range(ct):
                    eng.dma_start(xt[:, cc, :], xt_d[b, cc * P:(cc + 1) * P, :])
                return xt

            def emit_p1(b, x8):
                # phase 1: g1T = (A_sp @ x)^T; DoubleRow over k-tile pairs.
                # Pair-outer per cc over 4 parallel PSUM banks: one ldweights
                # per (cc, pair) serves all 4 quarters, and item-0 consumes
                # each at_sp (pair, quarter) as soon as its DMA lands.
                g1T = actp.tile([P, ct, npad], F8, tag="act", name=f"g1T_{b}")
                for cc in range(ct):
                    groups = [(psA.tile([P, 512], F32, tag="psA",
                                        name=f"ps1_{b}_{cc}_{q0}"), q0, qs)
                              for (q0, qs) in nqv0]
                    for k in range(kp):
                        for (ps, q0, qs) in groups:
                            nc.tensor.matmul(
                                ps[:, :qs],
                                lhsT=x8[:, 2 * k:2 * k + 2,
                                        cc * P:(cc + 1) * P],
                                rhs=at_sp[:, 2 * k:2 * k + 2, q0:q0 + qs],
                                start=(k == 0), stop=(k == kp - 1),
                                perf_mode=DR)
                    for (ps, q0, qs) in groups:
                        # DVE only: the scalar engine's queue is still
                        # issuing DMA descriptors when item 0-1 drain
                        nc.vector.tensor_copy(g1T[:, cc, q0:q0 + qs],
                                              ps[:, :qs])
                return g1T

            def emit_p2(b, g1T):
                # phase 2: a1T = relu(W1^T @ g1T + b1); single DoubleRow
                # instruction contracts both ci tiles
                a1T = actp.tile([P, ct, npad], F8, tag="act", name=f"a1T_{b}")
                for cc in range(ct):
                    # cols [n:npad] are read as phase-3 lhsT pads but never
                    # written by the trimmed quarters
                    nc.vector.memset(a1T[:, cc, n:npad], 0)
                for co in range(ct):
                    for (q0, qs) in nqv:
                        ps = psA.tile([P, 512], F32, tag="psA")
                        nc.tensor.matmul(
                            ps[:, :qs],
                            lhsT=w_sb[0][:, 0:2, co * P:(co + 1) * P],
                            rhs=g1T[:, 0:2, q0:q0 + qs],
                            start=True, stop=True, perf_mode=DR)
                        h = qs // 2
                        nc.scalar.activation(a1T[:, co, q0:q0 + h],
                                             ps[:, :h], RELU,
                                             bias=b1_sb[:, co:co + 1])
                        nc.vector.tensor_scalar(
                            a1T[:, co, q0 + h:q0 + qs], ps[:, h:qs],
                            b1_sb[:, co:co + 1], 0.0,
                            op0=mybir.AluOpType.add,
                            op1=mybir.AluOpType.max)
                return a1T

            def emit_p3(b, a1T):
                # phase 3: h2 = a1 @ W2 (natural layout)
                h2 = hp.tile([P, kt, c], F8, tag="h", name=f"h2_{b}")
                for k in range(kt):
                    ps = psW.tile([P, 512], F32, tag="psW")
                    nc.tensor.matmul(
                        ps[:, :c],
                        lhsT=a1T[:, 0:2, k * P:(k + 1) * P],
                        rhs=w_sb[1][:, 0:2, :],
                        start=True, stop=True, perf_mode=DR)
                    # alternate drains across DVE/Act so the copy chain
                    # keeps pace with the 256-col matmuls
                    if k % 2 == 0:
                        nc.vector.tensor_copy(h2[:, k, :], ps[:, :c])
                    else:
                        nc.scalar.copy(h2[:, k, :], ps[:, :c])
                return h2

            def emit_p4(b, h2):
                # phase 4: a2T = relu((A_tm @ h2)^T + b2); pair-outer per cc
                # (4-bank rounds) so item-0 consumes at_tm pairs as they land
                a2T = actp.tile([P, ct, npad], F8, tag="act", name=f"a2T_{b}")
                for cc in range(ct):
                    nc.vector.memset(a2T[:, cc, n:npad], 0)
                for cc in range(ct):
                    groups = [(psA.tile([P, 512], F32, tag="psA",
                                        name=f"ps4_{b}_{cc}_{q0}"), q0, qs)
                              for (q0, qs) in nqv]
                    for k in range(kp):
                        for (ps, q0, qs) in groups:
                            nc.tensor.matmul(
                                ps[:, :qs],
                                lhsT=h2[:, 2 * k:2 * k + 2,
                                        cc * P:(cc + 1) * P],
                                rhs=at_tm[:, 2 * k:2 * k + 2, q0:q0 + qs],
                                start=(k == 0), stop=(k == kp - 1),
                                perf_mode=DR)
                    for (ps, q0, qs) in groups:
                        # drain each quarter as two parallel halves
                        # (scalar + DVE) so the psA banks free ~2x sooner;
                        # the next item's phase 1 waits on these tiles
                        h = qs // 2
                        nc.scalar.activation(a2T[:, cc, q0:q0 + h],
                                             ps[:, :h], RELU,
                                             bias=b2_sb[:, cc:cc + 1])
                        nc.vector.tensor_scalar(
                            a2T[:, cc, q0 + h:q0 + qs], ps[:, h:qs],
                            b2_sb[:, cc:cc + 1], 0.0,
                            op0=mybir.AluOpType.add,
                            op1=mybir.AluOpType.max)
                return a2T

            def emit_p5(b, a2T):
                # phase 5: h3 = a2 @ W3; h3[row n] = b3
                h3 = hp.tile([P, kt, c], F8, tag="h", name=f"h3_{b}")
                for k in range(kt):
                    ps = psW.tile([P, 512], F32, tag="psW")
                    nc.tensor.matmul(
                        ps[:, :c],
                        lhsT=a2T[:, 0:2, k * P:(k + 1) * P],
                        rhs=w_sb[2][:, 0:2, :],
                        start=True, stop=True, perf_mode=DR)
                    if k % 2 == 0:
                        nc.vector.tensor_copy(h3[:, k, :], ps[:, :c])
                    else:
                        nc.scalar.copy(h3[:, k, :], ps[:, :c])
                nc.scalar.dma_start(
                    h3[bias_part:bias_part + 1, bias_tile, :], b3_d[:, :])
                return h3

            def emit_p6(b, xt, h3, last=False):
                # phase 6: outT = relu((A_sp @ h3)^T + xT), transposed-output
                # form (same shape as phase 1) -- one ldweights per
                # (cc, pair) serves 4 quarters. For the final item the
                # residual is accumulated on the PE via an identity matmul
                # so the drain (which nothing overlaps) is just relu+store.
                for cc in range(ct):
                    groups = [(psA.tile([P, 512], F32, tag="psA",
                                        name=f"ps6_{b}_{cc}_{q0}"), q0, qs)
                              for (q0, qs) in nqv]
                    for k in range(kp):
                        for (ps, q0, qs) in groups:
                            nc.tensor.matmul(
                                ps[:, :qs],
                                lhsT=h3[:, 2 * k:2 * k + 2,
                                        cc * P:(cc + 1) * P],
                                rhs=at_sp[:, 2 * k:2 * k + 2, q0:q0 + qs],
                                start=(k == 0),
                                stop=(k == kp - 1 and not last),
                                perf_mode=DR)
                    if last:
                        for (ps, q0, qs) in groups:
                            nc.tensor.matmul(
                                ps[:, :qs], lhsT=id_sb[:, :],
                                rhs=xt[:, cc, q0:q0 + qs],
                                start=False, stop=True)
                    for qi, (ps, q0, qs) in enumerate(groups):
                        ot = outp.tile([P, 512], BF16, tag="o")
                        if last:
                            nc.scalar.activation(ot[:, :qs], ps[:, :qs],
                                                 RELU)
                        else:
                            nc.vector.tensor_add(ot[:, :qs], ps[:, :qs],
                                                 xt[:, cc, q0:q0 + qs])
                            nc.scalar.activation(ot[:, :qs], ot[:, :qs],
                                                 RELU)
                        # spread the unoverlapped final stores across rings
                        eng = (nc.sync, nc.scalar, nc.gpsimd,
                               nc.sync)[qi % 4] if last else nc.sync
                        eng.dma_start(
                            out_d[b, cc * P:(cc + 1) * P, q0:q0 + qs],
                            ot[:, :qs])

            def emit_mid(b, g1T, xt, h3_to_p6_filler=None):
                a1T = emit_p2(b, g1T)
                h2 = emit_p3(b, a1T)
                a2T = emit_p4(b, h2)
                return emit_p5(b, a2T)

            # Emission order: x8_0's first pair enqueues ahead of at_sp on
            # the fast rings; item-1 phase 1 is hoisted between item-0
            # phase 1 and phase 2 so the PE has more matmul work queued
            # before the first at_tm use (its DMA trails at_sp). Each
            # item's phase 1 is emitted two items ahead so its matmuls can
            # fill PE gaps during the preceding items' drain-bound phases.
            x8 = {0: emit_load_x8(0)}
            emit_load_at(at_sp, atsp_d, split=True)
            emit_load_w_b()
            if bl > 1:
                x8[1] = emit_load_x8(1)
            g1T = {0: emit_p1(0, x8[0])}
            xt = {0: emit_load_xt(0)}
            if bl > 1:
                xt[1] = emit_load_xt(1)
                emit_load_at(at_tm, attm_d, split=False)
                g1T[1] = emit_p1(1, x8[1])
            else:
                emit_load_at(at_tm, attm_d, split=False)
            for b in range(bl):
                h3 = emit_mid(b, g1T.pop(b), xt[b])
                # item b+2's phase 1 sits between phase 5 and phase 6 in
                # the PE queue: it needs no fresh drains, so it bridges the
                # h3 drain latency that phase 6's ldweights waits on
                if b + 2 < bl:
                    x8[b + 2] = emit_load_x8(b + 2)
                    xt[b + 2] = emit_load_xt(b + 2)
                    g1T[b + 2] = emit_p1(b + 2, x8.pop(b + 2))
                emit_p6(b, xt.pop(b), h3, last=(b == bl - 1))

    nc.compile()
    return nc


def _norm_adj_T(edges, n, npad, bias_row):
    """A^T padded to [npad, npad] in fp32. AT[m, j] = A[j, m] where
    out[j] += A[j, m] * h[m]; edge (r -> c) contributes dinv[r]*dinv[c] at
    AT[r, c]. Self loops included. If bias_row, AT[n, :n] = 1 (bias fold)."""
    row = np.concatenate([edges[0], np.arange(n, dtype=np.int64)])
    col = np.concatenate([edges[1], np.arange(n, dtype=np.int64)])
    deg = np.bincount(col, minlength=n).astype(np.float32)
    dinv = np.zeros(n, np.float32)
    nz = deg > 0
    dinv[nz] = 1.0 / np.sqrt(deg[nz])
    norm = dinv[row] * dinv[col]
    at = np.zeros((npad, npad), np.float32)
    np.add.at(at, (row, col), norm)
    if bias_row:
        at[n, :n] = 1.0
    return at


def _tile_rows(a, kt):
    """[kt*P, F] -> [P, kt, F] so that [p, k, :] = a[k*P + p, :]."""
    return np.ascontiguousarray(
        a.reshape(kt, P, a.shape[-1]).transpose(1, 0, 2))


_PROGRAM_CACHE = {}


def _get_program(bl, n, c):
    key = (bl, n, c)
    if key not in _PROGRAM_CACHE:
        _PROGRAM_CACHE[key] = build_program(bl, n, c)
    return _PROGRAM_CACHE[key]


def run(inputs, trace=False, n_cores=N_CORES):
    x32 = np.asarray(inputs["x"], dtype=np.float32)
    xt = np.ascontiguousarray(x32.transpose(0, 2, 1)).astype(NP_BF16)
    npad_h = -(-(x32.shape[1] + 1) // P) * P
    x8 = np.zeros((x32.shape[0], npad_h, x32.shape[2]), NP_F8)
    x8[:, :x32.shape[1], :] = x32.astype(NP_F8)
    w1 = np.asarray(inputs["W1"], np.float32)
    w2 = np.asarray(inputs["W2"], np.float32)
    w3 = np.asarray(inputs["W3"], np.float32)
    b1 = np.asarray(inputs["b1"], np.float32)
    b2 = np.asarray(inputs["b2"], np.float32)
    b3 = np.asarray(inputs["b3"], np.float32)
    e_sp = np.asarray(inputs["keypoint_line_without_temporal"]).astype(np.int64)
    e_tm = np.asarray(inputs["keypoint_line_with_temporal"]).astype(np.int64)

    b_total, n, c = x32.shape
    bl = b_total // n_cores
    kt = -(-(n + 1) // P)
    npad = kt * P
    ct = c // P

    nc = _get_program(bl, n, c)

    npr = -(-n // 16) * 16
    at_sp = _tile_rows(
        _norm_adj_T(e_sp, n, npad, bias_row=True)[:, :npr].astype(NP_F8), kt)
    at_tm = _tile_rows(
        _norm_adj_T(e_tm, n, npad, bias_row=False)[:, :npr].astype(NP_F8), kt)
    shared = {
        "at_sp": at_sp,
        "at_tm": at_tm,
        "w1": _tile_rows(w1.astype(NP_F8), ct),
        "w2": _tile_rows(w2.astype(NP_F8), ct),
        "w3": _tile_rows(w3.astype(NP_F8), ct),
        "b1": np.ascontiguousarray(b1.reshape(ct, P).T),
        "b2": np.ascontiguousarray(b2.reshape(ct, P).T),
        "b3": np.ascontiguousarray(b3.astype(NP_F8)[None, :]),
        "ident": np.eye(P, dtype=NP_BF16),
    }
    in_maps = [
        {"xt": np.ascontiguousarray(xt[i * bl:(i + 1) * bl]),
         "x8": np.ascontiguousarray(x8[i * bl:(i + 1) * bl]), **shared}
        for i in range(n_cores)
    ]
    res = run_bass_kernel_spmd(nc, in_maps, core_ids=list(range(n_cores)),
                               trace=trace)
    out = np.concatenate(
        [np.asarray(r["out"]).astype(np.float32).transpose(0, 2, 1)
         for r in res.results], axis=0)
    return out, res


def kernel(**inputs) -> np.ndarray:
    out, _ = run(inputs, trace=False)
    return out
